# revision 1
# baseline (speedup 1.0000x reference)
"""Trainium2 Bass kernel: batched soft 3-SAT circuit evaluation.

out[b, c] = 1 - prod_k z[c,k],  z = (sign>0 ? 1-x : x)[idx],
x = sigmoid(emb[0]).  Every batch row is identical (input_idx is all
zeros, the embedding has a single row, and jnp.take clamps OOB), so the
device computes each clause result once and broadcast-writes the rows.

Sharding: clauses split across 8 NeuronCores (5250 each, padded 5376).
Host work is index-layout prep only (fold sign into a combined table
index, pad, order literals chunk-major, wrap into the 16-partition
GPSIMD gather layout) plus concatenation of per-core outputs.

Per-core device pipeline (H = 4 column chunks of 1344 cols):
  prologue (4 col-quarters, two HWDGE rings): broadcast-load emb row
    into raw[128, NV]; ACT sigmoid -> x table half; DVE (x*-1)+1 ->
    1-x table half.  Combined table tab[128, 2*NV].
  per chunk h:
    - GPSIMD ap_gather: z[128, 512] literals (8 Q7 groups x 168 clauses)
    - DVE: r = 1 - z0*z1*z2  [128, 168] (replicated within each
      16-partition group)
    - PE: per group g a [K=16]x[M=128]x[N=168] matmul with lhsT=1/16
      broadcasts group g's row into all 128 partitions of PSUM (bitwise
      exact: sum of 16 identical values * 1/16)
    - ACT: copy PSUM -> SBUF bcast tile [128, 8*168]
    - 8 row-block DMAs bcast -> out[128b:128b+128, 1344h:1344h+1344]
      (5.4KB descriptors), alternating the sync/scalar HWDGE rings.
"""

import numpy as np

NV = 10000
C_TOTAL = 42000
KLIT = 3
B = 1024
NCORES = 8
C_CORE = C_TOTAL // NCORES     # 5250
GROUPS = 8                     # Q7 cores / 16-partition groups
C_PAD = 5376                   # padded clauses per core
CPGS = [168, 168, 168, 84]     # clauses per (group, Q7-chunk)
H = len(CPGS)
C_CHUNKS = [8 * c for c in CPGS]          # output cols per Q7 chunk
C_OFFS = [sum(C_CHUNKS[:h]) for h in range(H)]
LPCS = [c * KLIT for c in CPGS]           # real literals per (g, chunk)
LPC_PADS = [-(-l // 32) * 32 for l in LPCS]   # pad to 32 (2-col align)
COLS_HS = [l // 16 for l in LPC_PADS]     # idx cols per chunk
COL_OFFS = [sum(COLS_HS[:h]) for h in range(H)]
IDX_COLS = sum(COLS_HS)
PBLK = 256                     # PSUM cols reserved per group block

# PE-gathered tail: the last 672 output cols are gathered on the tensor
# engine via one-hot radix matmuls while the Q7 cores work the rest.
PE_C = C_PAD - sum(C_CHUNKS)   # 672 clauses
PE_OFF = sum(C_CHUNKS)         # col offset 4704
PE_L = PE_C * KLIT             # 2016 literals
PE_LP = 2048                   # padded to 4 tiles of 512
PE_TILES = PE_LP // 512
RADIX = 128                    # idx' = 128*hi + lo; hi < 157, lo < 128

_CACHE = {}


def _build():
    import concourse.bass as bass
    import concourse.tile as tile
    from concourse import bacc, mybir
    from contextlib import ExitStack

    f32 = mybir.dt.float32
    AF = mybir.ActivationFunctionType
    OP = mybir.AluOpType

    nc = bacc.Bacc("TRN2", target_bir_lowering=False, debug=False,
                   num_devices=NCORES)
    emb_d = nc.dram_tensor("emb", [1, NV], f32, kind="ExternalInput")
    idx_d = nc.dram_tensor("idxw", [128, IDX_COLS], mybir.dt.int16,
                           kind="ExternalInput")
    hia_d = nc.dram_tensor("hia", [1, PE_LP], f32, kind="ExternalInput")
    hib_d = nc.dram_tensor("hib", [1, PE_LP], f32, kind="ExternalInput")
    lo_d = nc.dram_tensor("lo", [1, PE_LP], f32, kind="ExternalInput")
    out_d = nc.dram_tensor("out", [B, C_PAD], f32, kind="ExternalOutput")

    with tile.TileContext(nc) as tc, ExitStack() as ctx:
        const = ctx.enter_context(tc.tile_pool(name="const", bufs=1))
        work = ctx.enter_context(tc.tile_pool(name="work", bufs=2))
        psum = ctx.enter_context(
            tc.tile_pool(name="psum", bufs=1, space="PSUM"))
        pepsum = ctx.enter_context(
            tc.tile_pool(name="pepsum", bufs=2, space="PSUM"))
        dpool = ctx.enter_context(
            tc.tile_pool(name="dram", bufs=1, space="DRAM"))

        idx_sb = const.tile([128, IDX_COLS], mybir.dt.int16)

        # selector E[:, g, :]: E[k, g, m] = 1/16 iff k//16 == g; matmul
        # with it averages each group's 16 identical partition rows into
        # all 128 output partitions (bitwise exact).
        sel = const.tile([128, GROUPS, 128], f32)
        nc.vector.memset(sel[:], 1.0 / 16.0)
        # keep 1/16 only where 0 <= p - 16g <= 15, i.e. g == p//16
        nc.gpsimd.affine_select(sel[:, :, :], sel[:, :, :],
                                pattern=[[-16, GROUPS], [0, 128]],
                                compare_op=OP.is_ge, fill=0.0,
                                base=0, channel_multiplier=1)
        nc.gpsimd.affine_select(sel[:, :, :], sel[:, :, :],
                                pattern=[[16, GROUPS], [0, 128]],
                                compare_op=OP.is_ge, fill=0.0,
                                base=15, channel_multiplier=-1)

        # table padded to RADIX*157 = 20096 so the PE radix view is in
        # bounds; tail memset keeps the X2 copy finite
        tab = const.tile([128, 157 * RADIX], f32)
        nc.vector.memset(tab[:, 2 * NV:157 * RADIX], 0.0)
        rings = [nc.sync, nc.scalar]
        NQ = 8
        q = NV // NQ
        with tc.tile_pool(name="rawp", bufs=1) as rawp:
            raw = rawp.tile([128, NV], f32)
            # broadcast-load eighths alternate sync HWDGE / gpsimd
            # SWDGE: two queues give aggregate HBM-read rate, and the
            # scalar ring stays clear so ACT isn't delayed by dispatch
            for c in range(NQ):
                eng = nc.sync if c % 2 == 0 else nc.gpsimd
                eng.dma_start(
                    out=raw[:, c * q:(c + 1) * q],
                    in_=bass.AP(tensor=emb_d, offset=c * q,
                                ap=[[0, 128], [1, q]]))
            nc.gpsimd.dma_start(out=idx_sb[:], in_=idx_d[:, :])
            for c in range(NQ):
                sl = slice(c * q, (c + 1) * q)
                xs = slice(NV + c * q, NV + (c + 1) * q)
                nc.scalar.activation(tab[:, xs], raw[:, sl], AF.Sigmoid)
                # 1 - x on DVE, overlaps ACT of the next eighth
                nc.vector.tensor_scalar(tab[:, sl], tab[:, xs], -1.0,
                                        1.0, OP.mult, OP.add)

        # ---- PE-gather tail: one-hot inputs and table radix view ----
        hi_bc = const.tile([128, PE_LP], f32)
        hib_bc = const.tile([128, PE_LP], f32)
        lo_bc = const.tile([128, PE_LP], f32)
        for src_d, dst in ((hia_d, hi_bc), (hib_d, hib_bc), (lo_d, lo_bc)):
            nc.gpsimd.dma_start(
                out=dst[:],
                in_=bass.AP(tensor=src_d, offset=0,
                            ap=[[0, 128], [1, PE_LP]]))
        iota_i = const.tile([128, 1], mybir.dt.int32)
        nc.gpsimd.iota(iota_i[:], pattern=[[0, 1]], channel_multiplier=1)
        iota_f = const.tile([128, 1], f32)
        nc.vector.tensor_copy(iota_f[:], iota_i[:])
        ones_col = const.tile([128, 1], f32)
        nc.vector.memset(ones_col[:], 1.0)
        # one-hot masks per 512-literal tile (DVE, pre-gather window)
        oh_a, oh_b, oh_l = [], [], []
        for t in range(PE_TILES):
            sl = slice(512 * t, 512 * (t + 1))
            oa = const.tile([128, 512], f32, tag=f"oha{t}")
            nc.vector.tensor_scalar(oa[:], hi_bc[:, sl], iota_f[:, 0:1],
                                    None, OP.is_equal)
            ob = const.tile([128, 512], f32, tag=f"ohb{t}")
            nc.vector.tensor_scalar(ob[:], hib_bc[:, sl], iota_f[:, 0:1],
                                    None, OP.is_equal)
            ol = const.tile([128, 512], f32, tag=f"ohl{t}")
            nc.vector.tensor_scalar(ol[:], lo_bc[:, sl], iota_f[:, 0:1],
                                    None, OP.is_equal)
            oh_a.append(oa); oh_b.append(ob); oh_l.append(ol)
        # X2[k, m] = tab[128k + m] laid out across partitions
        x2a = const.tile([128, RADIX], f32)
        x2b = const.tile([29, RADIX], f32)
        tapr = tab[:].ap[0][0]
        nc.sync.dma_start(
            out=x2a[:],
            in_=bass.AP(tensor=tab[:].tensor, offset=tab[:].offset,
                        ap=[[tapr, 1], [1, 128 * RADIX]]))
        nc.sync.dma_start(
            out=x2b[:],
            in_=bass.AP(tensor=tab[:].tensor,
                        offset=tab[:].offset + 128 * RADIX,
                        ap=[[tapr, 1], [1, 29 * RADIX]]))
        # stage 1+2: Y = X2.T @ onehot_hi ; z = sum_p(Y * onehot_lo)
        zrow = const.tile([1, PE_LP], f32)
        for t in range(PE_TILES):
            Y = pepsum.tile([128, 512], f32, tag="Y")
            nc.tensor.matmul(Y[:], x2a[:], oh_a[t][:],
                             start=True, stop=False)
            nc.tensor.matmul(Y[:], x2b[:], oh_b[t][0:29, :],
                             start=False, stop=True)
            m_sb = work.tile([128, 512], f32, tag="msb")
            nc.vector.tensor_tensor(m_sb[:], Y[:], oh_l[t][:], OP.mult)
            zr = pepsum.tile([1, 512], f32, tag="zr")
            nc.tensor.matmul(zr[0:1, :], ones_col[:], m_sb[:],
                             start=True, stop=True)
            nc.scalar.activation(zrow[0:1, 512 * t:512 * (t + 1)],
                                 zr[0:1, :], AF.Copy)
        # products + (1 - .) on the single-partition row
        perow = const.tile([1, PE_C], f32)
        nc.vector.tensor_tensor(perow[0:1, :], zrow[0:1, 0:PE_L:3],
                                zrow[0:1, 1:PE_L:3], OP.mult)
        nc.vector.scalar_tensor_tensor(perow[0:1, :], perow[0:1, :], 1.0,
                                       zrow[0:1, 2:PE_L:3],
                                       OP.mult, OP.mult)
        nc.vector.tensor_scalar(perow[0:1, :], perow[0:1, :], -1.0, 1.0,
                                OP.mult, OP.add)
        # roundtrip through DRAM to broadcast across partitions
        drow = dpool.tile([1, PE_C], f32)
        nc.scalar.dma_start(out=drow[0:1, :], in_=perow[0:1, :])

        for h in range(H):
            CPG, LPC, LPC_PAD = CPGS[h], LPCS[h], LPC_PADS[h]
            C_CHUNK, C_OFF = C_CHUNKS[h], C_OFFS[h]
            z = work.tile([128, max(LPC_PADS)], f32, tag="z")
            nc.gpsimd.ap_gather(
                z[:, 0:LPC_PAD], tab[:],
                idx_sb[:, COL_OFFS[h]:COL_OFFS[h] + COLS_HS[h]],
                channels=128, num_elems=2 * NV, d=1, num_idxs=LPC_PAD)

            p01 = work.tile([128, max(CPGS)], f32, tag="p01")
            nc.vector.tensor_tensor(p01[:, 0:CPG], z[:, 0:LPC:3],
                                    z[:, 1:LPC:3], OP.mult)
            r = work.tile([128, max(CPGS)], f32, tag="r")
            # r = z0 z1 z2 (the 1 - . fold happens in the ACT copy)
            nc.vector.scalar_tensor_tensor(r[:, 0:CPG], p01[:, 0:CPG],
                                           1.0, z[:, 2:LPC:3],
                                           OP.mult, OP.mult)

            # PE broadcast: group g's (16-replicated) row -> all 128
            # partitions.  sum over the 16 identical values * 1/16 is
            # bitwise exact.
            P = psum.tile([128, GROUPS, PBLK], f32, tag="P")
            for g in range(GROUPS):
                nc.tensor.matmul(P[:, g, 0:CPG], sel[:, g, :],
                                 r[:, 0:CPG], start=True, stop=True)
            # pack the 8 group blocks contiguously so output descriptors
            # are C_CHUNK*4 bytes
            bcast = work.tile([128, GROUPS * max(CPGS)], f32, tag="bcast")
            bt = bcast[:]
            prow = bt.ap[0][0]
            bview = bass.AP(tensor=bt.tensor, offset=bt.offset,
                            ap=[[prow, 128], [CPG, GROUPS], [1, CPG]])
            # bcast = Copy(-P + 1) = 1 - z0 z1 z2
            nc.scalar.activation(bview, P[:, :, 0:CPG], AF.Copy,
                                 scale=-1.0, bias=1.0)

            out_w = C_CHUNK
            if h == H - 1:
                # append the PE-gathered tail columns via a stride-0
                # broadcast read of the DRAM row
                peb = bass.AP(tensor=bt.tensor, offset=bt.offset + C_CHUNK,
                              ap=[[prow, 128], [1, PE_C]])
                dr = drow[0:1, :]
                nc.scalar.dma_start(
                    out=peb,
                    in_=bass.AP(tensor=dr.tensor, offset=dr.offset,
                                ap=[[0, 128], [1, PE_C]]))
                out_w = C_CHUNK + PE_C

            # 8 row-block output DMAs, 128 rows each, spread across both
            # HWDGE rings
            bap = bass.AP(tensor=bt.tensor, offset=bt.offset,
                          ap=[[prow, 128], [1, out_w]])
            for blk in range(8):
                dst = bass.AP(tensor=out_d,
                              offset=blk * 128 * C_PAD + C_OFF,
                              ap=[[C_PAD, 128], [1, out_w]])
                rings[blk % 2].dma_start(out=dst, in_=bap)
    nc.compile()
    return nc


def _prep_indices(clause_idx, clause_sign):
    """Per-core wrapped int16 combined-index arrays [128, IDX_COLS].

    Literal order per group g: chunk-major — for chunk h, group g owns
    core clauses [C_CHUNK*h + CPG*g, C_CHUNK*h + CPG*(g+1)), padded to
    LPC_PAD literals per (group, chunk) block.
    """
    idx2 = clause_idx.astype(np.int32) + NV * (clause_sign <= 0.0)
    idx2 = idx2.astype(np.int16)
    per_core = []
    for c in range(NCORES):
        cl = idx2[c * C_CORE:(c + 1) * C_CORE]            # [5250, 3]
        buf = np.zeros((C_PAD, KLIT), dtype=np.int16)
        buf[:cl.shape[0]] = cl
        # group g's stream = concat over chunks of its padded block
        gs = np.zeros((GROUPS, IDX_COLS * 16), dtype=np.int16)
        for h in range(H):
            blk = buf[C_OFFS[h]:C_OFFS[h] + C_CHUNKS[h]]  # [8*CPG, 3]
            blk = blk.reshape(GROUPS, LPCS[h])
            o = COL_OFFS[h] * 16
            gs[:, o:o + LPCS[h]] = blk
        # wrap: literal j at partition 16g + j%16, col j//16
        w = (gs.reshape(GROUPS, IDX_COLS, 16)
               .transpose(0, 2, 1)
               .reshape(128, IDX_COLS))
        # PE tail: radix-decomposed literals, plain order, f32 rows
        pe = buf[PE_OFF:PE_OFF + PE_C].reshape(-1).astype(np.int32)
        pe = np.concatenate([pe, np.zeros(PE_LP - PE_L, np.int32)])
        hi = pe // RADIX
        hia = hi.astype(np.float32)[None, :]
        hib = (hi - 128).astype(np.float32)[None, :]
        lo = (pe % RADIX).astype(np.float32)[None, :]
        per_core.append((np.ascontiguousarray(w), hia, hib, lo))
    return per_core


def _ensure_ntff_hook():
    """The agent image lacks antenv.axon_hooks; synthesize it so
    run_bass_kernel_spmd(trace=True) can capture NTFF profiles."""
    import sys, types
    try:
        from antenv import axon_hooks  # noqa: F401
        return
    except ImportError:
        pass
    m = types.ModuleType("antenv.axon_hooks")
    _hook = [None]
    m.set_axon_ntff_profile_hook = lambda h: _hook.__setitem__(0, h)
    m.get_axon_ntff_profile_hook = lambda: _hook[0]
    sys.modules["antenv.axon_hooks"] = m
    import antenv
    antenv.axon_hooks = m
    from trn_agent_boot.trn_boot import _ntff_profile_via_ctypes
    m.set_axon_ntff_profile_hook(
        _ntff_profile_via_ctypes("/opt/axon/libaxon_pjrt.so"))


def _run(emb, idx_cores, trace=False):
    from concourse.bass_utils import run_bass_kernel_spmd
    if trace:
        _ensure_ntff_hook()
    if "prog" not in _CACHE:
        _CACHE["prog"] = _build()
    nc = _CACHE["prog"]
    in_maps = [{"emb": emb, "idxw": idx_cores[c][0],
                "hia": idx_cores[c][1], "hib": idx_cores[c][2],
                "lo": idx_cores[c][3]} for c in range(NCORES)]
    return run_bass_kernel_spmd(nc, in_maps, list(range(NCORES)),
                                trace=trace)


def kernel(input_idx=None, emb_weight=None, clause_idx=None,
           clause_sign=None, _trace=False, _want_results=False):
    emb = np.ascontiguousarray(np.asarray(emb_weight, dtype=np.float32))
    cidx = np.asarray(clause_idx, dtype=np.int32)
    csgn = np.asarray(clause_sign, dtype=np.float32)
    idx_cores = _prep_indices(cidx, csgn)
    res = _run(emb, idx_cores, trace=_trace)
    full = np.empty((B, C_TOTAL), dtype=np.float32)
    for c in range(NCORES):
        full[:, c * C_CORE:(c + 1) * C_CORE] = \
            res.results[c]["out"][:, :C_CORE]
    if _want_results:
        return full, res
    return full



# revision 7
# speedup vs baseline: 1.4659x; 1.4659x over previous
"""Trainium2 Bass kernel: batched soft 3-SAT circuit evaluation.

out[b, c] = 1 - prod_k z[c,k],  z_k = sigmoid(-s_k * w[i_k])   (uses
1 - sigmoid(w) = sigmoid(-w)), w = emb row, s = sign(clause_sign).
Every batch row is identical (input_idx is all zeros, the embedding has
a single row, jnp.take clamps OOB), so the device computes each clause
result once and broadcast-writes the rows in fp16 (rel err ~1e-3 vs
the 2e-2 gate); the host upcasts to f32.

Sharding: clauses split across 8 NeuronCores (5250 each, padded 5376).
Per core the 5376 clauses are processed by two parallel gather engines
(~10-15 ns/clause each):

- PE one-hot radix path: idx = 128*hi+lo; K=1 bf16 matmuls broadcast
  host-sent hi/lo rows to 128 partitions, DVE is_equal vs an iota
  column builds one-hots in bf16, stage-1 matmul X2[80,128] x oh_hi
  gathers w into Y[128,512], DVE masks with oh_lo, stage-2 matmuls
  with a column-selector lhsT accumulate tile t's literals into row t
  of a PSUM block.  DVE sign-mult + ACT sigmoid + DVE products ->
  r2[4,168], row-selector matmuls broadcast to 128 partitions.
  Tiles are software-pipelined with double-buffered Phi/Plo PSUM.
- GPSIMD ap_gather path: gathers w f32 from a [128, NV] broadcast
  table (the only large load, 2.56 MB, all four quarters on the SWDGE
  ring), DVE sign-mult, ACT sigmoid, DVE products, 1/16-selector
  matmuls broadcast each group's row (bitwise exact: 16 identical
  values * 1/16).

All selector/index constants ship in ONE packed [128, 2560] int16
tensor (5 KB descriptors — HWDGE descriptor processing is ~40ns each,
so many small loads would serialize for ~20us).  No gpsimd iota or
affine_select (any extended Q7 instruction other than ap_gather forces
a ~16us ucode library swap).  Every matmul output block sits inside a
single 2KB PSUM bank.  ACT copies fold 1 - r while casting to fp16;
output DMAs alternate the two HWDGE rings.
"""

import numpy as np

NV = 10000
C_TOTAL = 42000
KLIT = 3
B = 1024
NCORES = 8
C_CORE = C_TOTAL // NCORES     # 5250
GROUPS = 8
C_PAD = 5376
TILE_C = 168                   # clauses per PE tile (504 lits pad 512)
RADIX = 128                    # idx = 128*hi + lo; hi < 79

# chunk plan: emission order == output column order
# ('pe', clauses) with clauses <= 4*TILE_C (multiple of TILE_C), or
# ('gp', clauses) with clauses % 8 == 0 (CPG = clauses/8 <= 256)
PLAN = [('pe', 336), ('pe', 672), ('pe', 672), ('pe', 672),
        ('gp', 1256), ('pe', 504), ('gp', 1264)]
assert sum(c for _, c in PLAN) == C_PAD

PE_CHUNKS = [(i, c) for i, (k, c) in enumerate(PLAN) if k == 'pe']
GP_CHUNKS = [(i, c) for i, (k, c) in enumerate(PLAN) if k == 'gp']
COL_OFFS = np.concatenate([[0], np.cumsum([c for _, c in PLAN])]).tolist()


def _ntiles(c):
    return -(-c // TILE_C)


PE_NTILES = [_ntiles(c) for _, c in PE_CHUNKS]
PE_TILES_TOT = sum(PE_NTILES)


def _gp_geom(c):
    cpg = c // GROUPS
    lpc = cpg * KLIT
    lpc_pad = -(-lpc // 32) * 32
    return cpg, lpc, lpc_pad


GP_GEOM = [_gp_geom(c) for _, c in GP_CHUNKS]
GP_IDX_COLS = [lp // 16 for _, _, lp in GP_GEOM]
IDX_COLS = sum(GP_IDX_COLS)
GP_COL_OFFS = np.concatenate([[0], np.cumsum(GP_IDX_COLS)]).tolist()
SGN_TOT = sum(lp for _, _, lp in GP_GEOM)

# packed per-core constants, bf16 [128, PAUX_COLS]:
#   sel [128,8,128] | colsel [128,4,128] | sgnz | iota [128,1] | pad
# (indices ship separately as int16 on the SWDGE ring)
PAUX_SEL = 0
PAUX_CSEL = PAUX_SEL + GROUPS * 128
PAUX_SGN = PAUX_CSEL + 4 * 128
PAUX_IOTA = PAUX_SGN + SGN_TOT
PAUX_COLS = -(-(PAUX_IOTA + 1) // 32) * 32

_CACHE = {}


def _build():
    import concourse.bass as bass
    import concourse.tile as tile
    from concourse import bacc, mybir
    from contextlib import ExitStack

    f32 = mybir.dt.float32
    f16 = mybir.dt.float16
    bf16 = mybir.dt.bfloat16
    i16 = mybir.dt.int16
    AF = mybir.ActivationFunctionType
    OP = mybir.AluOpType

    nc = bacc.Bacc("TRN2", target_bir_lowering=False, debug=False,
                   num_devices=NCORES)
    emb_d = nc.dram_tensor("emb", [1, NV], f32, kind="ExternalInput")
    paux_d = nc.dram_tensor("paux", [128, PAUX_COLS], bf16,
                            kind="ExternalInput")
    pidx_d = nc.dram_tensor("pidx", [128, IDX_COLS], i16,
                            kind="ExternalInput")
    perows_d = nc.dram_tensor("perows", [1, PE_TILES_TOT * 1024], bf16,
                              kind="ExternalInput")
    smalls_d = nc.dram_tensor("smalls", [4, len(PE_CHUNKS) * 512 + 512],
                              bf16, kind="ExternalInput")
    out_d = nc.dram_tensor("out", [B, C_PAD], f16, kind="ExternalOutput")

    with tile.TileContext(nc) as tc, ExitStack() as ctx:
        const = ctx.enter_context(tc.tile_pool(name="const", bufs=1))
        work = ctx.enter_context(tc.tile_pool(name="work", bufs=2))
        ymp = ctx.enter_context(tc.tile_pool(name="ymp", bufs=4))
        psum = ctx.enter_context(
            tc.tile_pool(name="psum", bufs=1, space="PSUM"))

        # PSUM: PA(2 banks) zP PhiA PhiB PloA PloB Y = 8 banks exactly
        PA = psum.tile([128, 4, 256], f32, tag="PA")
        zP = psum.tile([128, 512], f32, tag="zP")
        Phis = [psum.tile([128, 512], f32, tag="phiA", name="phiA"),
                psum.tile([128, 512], f32, tag="phiB", name="phiB")]
        Plos = [psum.tile([128, 512], f32, tag="ploA", name="ploA"),
                psum.tile([128, 512], f32, tag="ploB", name="ploB")]
        Y = psum.tile([128, 512], f32, tag="Y")

        # ---- loads -------------------------------------------------
        # scalar ring: perows (1 desc), smalls (4), x2 (79) — all tiny
        # sync ring: paux (one 5KB-desc DMA)
        # gpsimd SWDGE: the four tab quarters (10KB descs, fast ring)
        perows = const.tile([1, PE_TILES_TOT * 1024], bf16)
        nc.scalar.dma_start(out=perows[:], in_=perows_d[:, :])
        smalls = const.tile([4, len(PE_CHUNKS) * 512 + 512], bf16)
        nc.scalar.dma_start(out=smalls[:], in_=smalls_d[:, :])
        x2 = const.tile([80, 128], f32)
        nc.vector.memset(x2[:], 0.0)
        nc.scalar.dma_start(
            out=x2[0:78, :],
            in_=bass.AP(tensor=emb_d, offset=0, ap=[[128, 78], [1, 128]]))
        nc.scalar.dma_start(
            out=x2[78:79, 0:16],
            in_=bass.AP(tensor=emb_d, offset=9984, ap=[[16, 1], [1, 16]]))

        paux = const.tile([128, PAUX_COLS], bf16)
        nc.sync.dma_start(out=paux[:], in_=paux_d[:, :])
        pidx = const.tile([128, IDX_COLS], i16)
        nc.gpsimd.dma_start(out=pidx[:], in_=pidx_d[:, :])
        pt = paux[:]
        prow_x = pt.ap[0][0]

        def paux_mat(off):
            # [128, 128] lhsT view at bf16 column offset `off`
            return bass.AP(tensor=pt.tensor, offset=pt.offset + off,
                           ap=[[prow_x, 128], [1, 128]])

        iota_bv = bass.AP(tensor=pt.tensor, offset=pt.offset + PAUX_IOTA,
                          ap=[[prow_x, 128], [1, 1]])

        tab = const.tile([128, NV], f32)
        q = NV // 4
        for c in range(4):
            nc.gpsimd.dma_start(
                out=tab[:, c * q:(c + 1) * q],
                in_=bass.AP(tensor=emb_d, offset=c * q,
                            ap=[[0, 128], [1, q]]))

        x2b = const.tile([80, 128], bf16)
        nc.vector.tensor_copy(x2b[:], x2[:])
        ones1 = const.tile([1, 128], bf16)
        nc.vector.memset(ones1[:], 1.0)
        iota = const.tile([128, 1], f32)
        nc.scalar.activation(iota[:], iota_bv, AF.Copy)

        # ---- GP gathers issued early in the gpsimd stream ----------
        gp_z = []
        for gi, (ci, c) in enumerate(GP_CHUNKS):
            cpg, lpc, lpc_pad = GP_GEOM[gi]
            z = const.tile([128, lpc_pad], f32, tag=f"z{gi}")
            nc.gpsimd.ap_gather(
                z[:], tab[:],
                pidx[:, GP_COL_OFFS[gi]:GP_COL_OFFS[gi] + GP_IDX_COLS[gi]],
                channels=128, num_elems=NV, d=1, num_idxs=lpc_pad)
            gp_z.append(z)

        rings = [nc.sync, nc.scalar]

        def write_out(bcast, ci, cols):
            bt = bcast[:]
            prow = bt.ap[0][0]
            bap = bass.AP(tensor=bt.tensor, offset=bt.offset,
                          ap=[[prow, 128], [1, cols]])
            for blk in range(8):
                dst = bass.AP(tensor=out_d,
                              offset=blk * 128 * C_PAD + COL_OFFS[ci],
                              ap=[[C_PAD, 128], [1, cols]])
                rings[blk % 2].dma_start(out=dst, in_=bap)

        # pending tensor-tail from the previous chunk (bcast matmuls +
        # ACT copy + DMAs), emitted after the next chunk's first
        # tensor ops so the PE never idles at chunk boundaries
        pending = []

        def flush_pending():
            while pending:
                pending.pop(0)()

        gtile = 0
        pi = 0
        gi = 0
        for ci, (kind, c) in enumerate(PLAN):
            if kind == 'pe':
                nt = _ntiles(c)
                ohs = []
                yms = []

                def stage1(t, ohs=ohs, yms=yms):
                    nc.tensor.matmul(Y[:], x2b[:], ohs[t][0][0:80, :],
                                     start=True, stop=True)
                    ym = ymp.tile([128, 512], bf16, tag="ym")
                    nc.vector.tensor_tensor(ym[:], Y[:], ohs[t][1][:],
                                            OP.mult)
                    yms.append(ym)

                for t in range(nt):
                    hirow = perows[0:1, 1024 * gtile:1024 * gtile + 512]
                    lorow = perows[0:1,
                                   1024 * gtile + 512:1024 * (gtile + 1)]
                    gtile += 1
                    Pht, Plt = Phis[t % 2], Plos[t % 2]
                    nc.tensor.matmul(Pht[:], ones1[:], hirow,
                                     start=True, stop=True)
                    nc.tensor.matmul(Plt[:], ones1[:], lorow,
                                     start=True, stop=True)
                    if t == 0:
                        flush_pending()
                    oh_hi = work.tile([128, 512], bf16, tag="ohhi")
                    nc.vector.tensor_scalar(oh_hi[:], Pht[:],
                                            iota[:, 0:1], None,
                                            OP.is_equal)
                    oh_lo = work.tile([128, 512], bf16, tag="ohlo")
                    nc.vector.tensor_scalar(oh_lo[:], Plt[:],
                                            iota[:, 0:1], None,
                                            OP.is_equal)
                    ohs.append((oh_hi, oh_lo))
                    if t >= 1:
                        stage1(t - 1)
                stage1(nt - 1)
                for t in range(nt):
                    nc.tensor.matmul(zP[:], paux_mat(PAUX_CSEL + 128 * t),
                                     yms[t][:],
                                     start=(t == 0), stop=(t == nt - 1))
                zsg = work.tile([4, 512], f32, tag="zsg")
                nc.vector.tensor_tensor(
                    zsg[0:nt, :], zP[0:nt, :],
                    smalls[0:nt, 512 * pi:512 * (pi + 1)], OP.mult)
                zs = work.tile([4, 512], f32, tag="zs")
                nc.scalar.activation(zs[0:nt, :], zsg[0:nt, :],
                                     AF.Sigmoid)
                p01 = work.tile([4, TILE_C], f32, tag="pp01")
                nc.vector.tensor_tensor(p01[0:nt, :], zs[0:nt, 0:504:3],
                                        zs[0:nt, 1:504:3], OP.mult)
                r2 = work.tile([4, TILE_C], bf16, tag="pr2")
                nc.vector.scalar_tensor_tensor(r2[0:nt, :], p01[0:nt, :],
                                               1.0, zs[0:nt, 2:504:3],
                                               OP.mult, OP.mult)
                rsel_o = len(PE_CHUNKS) * 512

                def tail(ci=ci, c=c, nt=nt, r2=r2):
                    for t in range(nt):
                        nc.tensor.matmul(
                            PA[:, t, 0:TILE_C],
                            smalls[0:nt, rsel_o + 128 * t:
                                   rsel_o + 128 * (t + 1)],
                            r2[0:nt, :], start=True, stop=True)
                    bcast = const.tile([128, c], f16, tag=f"bc{ci}")
                    bt = bcast[:]
                    pav = PA[:]
                    nc.scalar.activation(
                        bass.AP(tensor=bt.tensor, offset=bt.offset,
                                ap=[[bt.ap[0][0], 128], [TILE_C, nt],
                                    [1, TILE_C]]),
                        bass.AP(tensor=pav.tensor, offset=pav.offset,
                                ap=[[pav.ap[0][0], 128], [256, nt],
                                    [1, TILE_C]]),
                        AF.Copy, scale=-1.0, bias=1.0)
                    write_out(bcast, ci, c)
                pending.append(tail)
                pi += 1
            else:
                cpg, lpc, lpc_pad = GP_GEOM[gi]
                z = gp_z[gi]
                o = sum(lp for _, _, lp in GP_GEOM[:gi])
                flush_pending()
                zsg = work.tile([128, lpc_pad], f32, tag="gzsg")
                nc.vector.tensor_tensor(
                    zsg[:], z[:],
                    bass.AP(tensor=pt.tensor,
                            offset=pt.offset + PAUX_SGN + o,
                            ap=[[prow_x, 128], [1, lpc_pad]]), OP.mult)
                zs = work.tile([128, lpc_pad], f32, tag="gzs")
                nc.scalar.activation(zs[:], zsg[:], AF.Sigmoid)
                p01 = work.tile([128, cpg], f32, tag="gp01")
                nc.vector.tensor_tensor(p01[:], zs[:, 0:lpc:3],
                                        zs[:, 1:lpc:3], OP.mult)
                r = work.tile([128, cpg], bf16, tag="gr")
                nc.vector.scalar_tensor_tensor(r[:], p01[:], 1.0,
                                               zs[:, 2:lpc:3],
                                               OP.mult, OP.mult)
                bcast = const.tile([128, c], f16, tag=f"bc{ci}")
                bt = bcast[:]
                prow = bt.ap[0][0]
                pav = PA[:]
                for half in range(2):
                    for g4 in range(4):
                        g = 4 * half + g4
                        nc.tensor.matmul(PA[:, g4, 0:cpg],
                                         paux_mat(PAUX_SEL + 128 * g),
                                         r[:], start=True, stop=True)
                    nc.scalar.activation(
                        bass.AP(tensor=bt.tensor,
                                offset=bt.offset + half * 4 * cpg,
                                ap=[[prow, 128], [cpg, 4], [1, cpg]]),
                        bass.AP(tensor=pav.tensor, offset=pav.offset,
                                ap=[[pav.ap[0][0], 128], [256, 4],
                                    [1, cpg]]),
                        AF.Copy, scale=-1.0, bias=1.0)
                write_out(bcast, ci, c)
                gi += 1
        flush_pending()
    nc.compile()
    return nc


def _prep_inputs(clause_idx, clause_sign):
    import ml_dtypes
    bf = ml_dtypes.bfloat16
    idx_all = clause_idx.astype(np.int32)
    # product factor per literal is (1 - y) = sigmoid(-sign * w): the
    # sign fed to the device sigmoid is the NEGATED clause sign
    sgn_all = np.where(clause_sign > 0.0, np.float32(-1.0),
                       np.float32(1.0))

    # shared constant blocks (same every core)
    k = np.arange(128)
    sel = (((k[:, None] // 16) == np.arange(GROUPS)[None, :])
           .astype(np.float32)[:, :, None]
           * np.full((1, 1, 128), 1.0 / 16.0, np.float32))
    sel = np.ascontiguousarray(np.broadcast_to(sel, (128, GROUPS, 128)))
    colsel = np.zeros((128, 4, 128), dtype=np.float32)
    for t in range(4):
        colsel[:, t, t] = 1.0
    rowsel = np.zeros((4, 4, 128), dtype=np.float32)
    for t in range(4):
        rowsel[t, t, :] = 1.0
    iota = np.arange(128, dtype=np.float32)

    per_core = []
    for cc in range(NCORES):
        cl_i = np.zeros((C_PAD, KLIT), dtype=np.int32)
        cl_s = np.ones((C_PAD, KLIT), dtype=np.float32)
        cl_i[:C_CORE] = idx_all[cc * C_CORE:(cc + 1) * C_CORE]
        cl_s[:C_CORE] = sgn_all[cc * C_CORE:(cc + 1) * C_CORE]

        perows = np.zeros((1, PE_TILES_TOT * 1024), dtype=np.float32)
        smalls = np.ones((4, len(PE_CHUNKS) * 512 + 512),
                         dtype=np.float32)
        smalls[:, len(PE_CHUNKS) * 512:] = rowsel.reshape(4, 512)
        idxw = np.zeros((128, IDX_COLS), dtype=np.int16)
        sgnz = np.ones((128, SGN_TOT), dtype=np.float32)

        gtile = 0
        pi = 0
        gi = 0
        for ci, (kind, c) in enumerate(PLAN):
            c0 = COL_OFFS[ci]
            if kind == 'pe':
                nt = _ntiles(c)
                for t in range(nt):
                    ncl = min(TILE_C, c - TILE_C * t)
                    ii = np.zeros((TILE_C, KLIT), dtype=np.int32)
                    ss = np.ones((TILE_C, KLIT), dtype=np.float32)
                    ii[:ncl] = cl_i[c0 + TILE_C * t:c0 + TILE_C * t + ncl]
                    ss[:ncl] = cl_s[c0 + TILE_C * t:c0 + TILE_C * t + ncl]
                    lits = ii.reshape(-1)
                    srow = ss.reshape(-1)
                    hi = (lits // RADIX).astype(np.float32)
                    lo = (lits % RADIX).astype(np.float32)
                    perows[0, 1024 * gtile:1024 * gtile + 504] = hi
                    perows[0, 1024 * gtile + 512:1024 * gtile + 1016] = lo
                    smalls[t, 512 * pi:512 * pi + 504] = srow
                    gtile += 1
                pi += 1
            else:
                cpg, lpc, lpc_pad = GP_GEOM[gi]
                blk_i = cl_i[c0:c0 + c].reshape(GROUPS, lpc)
                blk_s = cl_s[c0:c0 + c].reshape(GROUPS, lpc)
                gs_i = np.zeros((GROUPS, lpc_pad), dtype=np.int32)
                gs_s = np.ones((GROUPS, lpc_pad), dtype=np.float32)
                gs_i[:, :lpc] = blk_i
                gs_s[:, :lpc] = blk_s
                wi = (gs_i.reshape(GROUPS, lpc_pad // 16, 16)
                      .transpose(0, 2, 1).reshape(128, lpc_pad // 16))
                idxw[:, GP_COL_OFFS[gi]:GP_COL_OFFS[gi] +
                     GP_IDX_COLS[gi]] = wi
                o = sum(lp for _, _, lp in GP_GEOM[:gi])
                sgnz[:, o:o + lpc_pad] = np.repeat(
                    gs_s[:, None, :], 16, axis=1).reshape(128, lpc_pad)
                gi += 1

        paux = np.zeros((128, PAUX_COLS), dtype=np.float32)
        paux[:, PAUX_SEL:PAUX_CSEL] = sel.reshape(128, -1)
        paux[:, PAUX_CSEL:PAUX_SGN] = colsel.reshape(128, -1)
        paux[:, PAUX_SGN:PAUX_IOTA] = sgnz
        paux[:, PAUX_IOTA] = iota

        per_core.append({
            "paux": paux.astype(bf),
            "pidx": idxw,
            "perows": perows.astype(bf),
            "smalls": smalls.astype(bf),
        })
    return per_core


def _ensure_ntff_hook():
    """The agent image lacks antenv.axon_hooks; synthesize it so
    run_bass_kernel_spmd(trace=True) can capture NTFF profiles."""
    import sys, types
    try:
        from antenv import axon_hooks  # noqa: F401
        return
    except ImportError:
        pass
    m = types.ModuleType("antenv.axon_hooks")
    _hook = [None]
    m.set_axon_ntff_profile_hook = lambda h: _hook.__setitem__(0, h)
    m.get_axon_ntff_profile_hook = lambda: _hook[0]
    sys.modules["antenv.axon_hooks"] = m
    import antenv
    antenv.axon_hooks = m
    from trn_agent_boot.trn_boot import _ntff_profile_via_ctypes
    m.set_axon_ntff_profile_hook(
        _ntff_profile_via_ctypes("/opt/axon/libaxon_pjrt.so"))


def _run(emb, per_core, trace=False):
    from concourse.bass_utils import run_bass_kernel_spmd
    if trace:
        _ensure_ntff_hook()
    if "prog" not in _CACHE:
        _CACHE["prog"] = _build()
    nc = _CACHE["prog"]
    in_maps = [{"emb": emb, **per_core[c]} for c in range(NCORES)]
    return run_bass_kernel_spmd(nc, in_maps, list(range(NCORES)),
                                trace=trace)


def kernel(input_idx=None, emb_weight=None, clause_idx=None,
           clause_sign=None, _trace=False, _want_results=False):
    emb = np.ascontiguousarray(np.asarray(emb_weight, dtype=np.float32))
    cidx = np.asarray(clause_idx, dtype=np.int32)
    csgn = np.asarray(clause_sign, dtype=np.float32)
    per_core = _prep_inputs(cidx, csgn)
    res = _run(emb, per_core, trace=_trace)
    full = np.empty((B, C_TOTAL), dtype=np.float32)
    for c in range(NCORES):
        full[:, c * C_CORE:(c + 1) * C_CORE] = \
            res.results[c]["out"][:, :C_CORE].astype(np.float32)
    if _want_results:
        return full, res
    return full


# revision 10
# speedup vs baseline: 1.5474x; 1.0556x over previous
"""Trainium2 Bass kernel: batched soft 3-SAT circuit evaluation.

out[b, c] = 1 - prod_k z[c,k],  z_k = sigmoid(-s_k * w[i_k])   (uses
1 - sigmoid(w) = sigmoid(-w)), w = emb row, s = sign(clause_sign).
Every batch row is identical (input_idx is all zeros, the embedding has
a single row, jnp.take clamps OOB), so the device computes each clause
result once and broadcast-writes the rows in fp16 (rel err ~1e-3 vs
the 2e-2 gate); the host upcasts to f32.

Sharding: clauses split across 8 NeuronCores (5250 each, padded 5376).
Per core the 5376 clauses are processed by two parallel gather engines
(~10-15 ns/clause each):

- PE one-hot radix path: idx = 128*hi+lo; K=1 bf16 matmuls broadcast
  host-sent hi/lo rows to 128 partitions, DVE is_equal vs an iota
  column builds one-hots in bf16, stage-1 matmul X2[80,128] x oh_hi
  gathers w into Y[128,512], DVE masks with oh_lo, stage-2 matmuls
  with a column-selector lhsT accumulate tile t's literals into row t
  of a PSUM block.  DVE sign-mult + ACT sigmoid + DVE products ->
  r2[4,168], row-selector matmuls broadcast to 128 partitions.
  Tiles are software-pipelined with double-buffered Phi/Plo PSUM.
- GPSIMD ap_gather path: gathers w f32 from a [128, NV] broadcast
  table (the only large load, 2.56 MB, all four quarters on the SWDGE
  ring), DVE sign-mult, ACT sigmoid, DVE products, 1/16-selector
  matmuls broadcast each group's row (bitwise exact: 16 identical
  values * 1/16).

All selector/index constants ship in ONE packed [128, 2560] int16
tensor (5 KB descriptors — HWDGE descriptor processing is ~40ns each,
so many small loads would serialize for ~20us).  No gpsimd iota or
affine_select (any extended Q7 instruction other than ap_gather forces
a ~16us ucode library swap).  Every matmul output block sits inside a
single 2KB PSUM bank.  ACT copies fold 1 - r while casting to fp16;
output DMAs alternate the two HWDGE rings.
"""

import numpy as np

NV = 10000
C_TOTAL = 42000
KLIT = 3
B = 1024
NCORES = 8
C_CORE = C_TOTAL // NCORES     # 5250
GROUPS = 8
C_PAD = 5376
TILE_C = 168                   # clauses per PE tile (504 lits pad 512)
RADIX = 128                    # idx = 128*hi + lo; hi < 79

# chunk plan: emission order == output column order
# ('pe', clauses) with clauses <= 4*TILE_C (multiple of TILE_C), or
# ('gp', clauses) with clauses % 8 == 0 (CPG = clauses/8 <= 256)
PLAN = [('pe', 336), ('pe', 672), ('gp', 1008), ('pe', 672),
        ('gp', 1008), ('pe', 672), ('gp', 1008)]
assert sum(c for _, c in PLAN) == C_PAD

PE_CHUNKS = [(i, c) for i, (k, c) in enumerate(PLAN) if k == 'pe']
GP_CHUNKS = [(i, c) for i, (k, c) in enumerate(PLAN) if k == 'gp']
COL_OFFS = np.concatenate([[0], np.cumsum([c for _, c in PLAN])]).tolist()


def _ntiles(c):
    return -(-c // TILE_C)


PE_NTILES = [_ntiles(c) for _, c in PE_CHUNKS]
PE_TILES_TOT = sum(PE_NTILES)


def _gp_geom(c):
    cpg = c // GROUPS
    lpc = cpg * KLIT
    lpc_pad = -(-lpc // 32) * 32
    return cpg, lpc, lpc_pad


GP_GEOM = [_gp_geom(c) for _, c in GP_CHUNKS]
GP_IDX_COLS = [lp // 16 for _, _, lp in GP_GEOM]
IDX_COLS = sum(GP_IDX_COLS)
GP_COL_OFFS = np.concatenate([[0], np.cumsum(GP_IDX_COLS)]).tolist()
SGN_TOT = sum(lp for _, _, lp in GP_GEOM)

# packed per-core constants, bf16 [128, PAUX_COLS]:
#   sel [128,8,128] | colsel [128,4,128] | sgnz | iota [128,1] | pad
# (indices ship separately as int16 on the SWDGE ring)
PAUX_SEL = 0
PAUX_CSEL = PAUX_SEL + GROUPS * 128
PAUX_SGN = PAUX_CSEL + 4 * 128
PAUX_IOTA = PAUX_SGN + SGN_TOT
PAUX_COLS = -(-(PAUX_IOTA + 1) // 32) * 32
# pidx int16 [128, IDX_COLS + SGN_TOT]: pair indices | parity (0/1)
PIDX_PAR = IDX_COLS

_CACHE = {}


def _build():
    import concourse.bass as bass
    import concourse.tile as tile
    from concourse import bacc, mybir
    from contextlib import ExitStack

    f32 = mybir.dt.float32
    f16 = mybir.dt.float16
    bf16 = mybir.dt.bfloat16
    i16 = mybir.dt.int16
    AF = mybir.ActivationFunctionType
    OP = mybir.AluOpType

    nc = bacc.Bacc("TRN2", target_bir_lowering=False, debug=False,
                   num_devices=NCORES)
    emb_d = nc.dram_tensor("emb", [1, NV], f32, kind="ExternalInput")
    paux_d = nc.dram_tensor("paux", [128, PAUX_COLS], bf16,
                            kind="ExternalInput")
    pidx_d = nc.dram_tensor("pidx", [128, IDX_COLS + SGN_TOT], i16,
                            kind="ExternalInput")
    perows_d = nc.dram_tensor("perows", [1, PE_TILES_TOT * 1024], bf16,
                              kind="ExternalInput")
    smalls_d = nc.dram_tensor("smalls", [4, len(PE_CHUNKS) * 512 + 512],
                              bf16, kind="ExternalInput")
    out_d = nc.dram_tensor("out", [B, C_PAD], f16, kind="ExternalOutput")

    with tile.TileContext(nc) as tc, ExitStack() as ctx:
        const = ctx.enter_context(tc.tile_pool(name="const", bufs=1))
        work = ctx.enter_context(tc.tile_pool(name="work", bufs=2))
        ymp = ctx.enter_context(tc.tile_pool(name="ymp", bufs=4))
        psum = ctx.enter_context(
            tc.tile_pool(name="psum", bufs=1, space="PSUM"))

        # PSUM: PA(2) zP(1) PPa(2) PPb(2) Y(1) = 8 banks exactly
        PA = psum.tile([128, 4, 256], f32, tag="PA")
        zP = psum.tile([128, 512], f32, tag="zP")
        PPs = [psum.tile([128, 1024], f32, tag="ppA", name="ppA"),
               psum.tile([128, 1024], f32, tag="ppB", name="ppB")]
        Y = psum.tile([128, 512], f32, tag="Y")

        # ---- loads -------------------------------------------------
        # scalar ring: perows (1 desc), smalls (4), x2 (79) — all tiny
        # sync ring: paux (one 5KB-desc DMA)
        # gpsimd SWDGE: the four tab quarters (10KB descs, fast ring)
        perows = const.tile([1, PE_TILES_TOT * 1024], bf16)
        nc.scalar.dma_start(out=perows[:], in_=perows_d[:, :])
        x2 = const.tile([80, 128], f32)
        nc.vector.memset(x2[:], 0.0)
        nc.scalar.dma_start(
            out=x2[0:78, :],
            in_=bass.AP(tensor=emb_d, offset=0, ap=[[128, 78], [1, 128]]))
        nc.scalar.dma_start(
            out=x2[78:79, 0:16],
            in_=bass.AP(tensor=emb_d, offset=9984, ap=[[16, 1], [1, 16]]))
        smalls = const.tile([4, len(PE_CHUNKS) * 512 + 512], bf16)
        nc.scalar.dma_start(out=smalls[:], in_=smalls_d[:, :])

        paux = const.tile([128, PAUX_COLS], bf16)
        nc.sync.dma_start(out=paux[:], in_=paux_d[:, :])
        pidx = const.tile([128, IDX_COLS + SGN_TOT], i16)
        nc.gpsimd.dma_start(out=pidx[:], in_=pidx_d[:, :])
        pt = paux[:]
        prow_x = pt.ap[0][0]

        def paux_mat(off):
            # [128, 128] lhsT view at bf16 column offset `off`
            return bass.AP(tensor=pt.tensor, offset=pt.offset + off,
                           ap=[[prow_x, 128], [1, 128]])

        iota_bv = bass.AP(tensor=pt.tensor, offset=pt.offset + PAUX_IOTA,
                          ap=[[prow_x, 128], [1, 1]])

        x2b = const.tile([80, 128], bf16)
        nc.vector.tensor_copy(x2b[:], x2[:])
        ones1 = const.tile([1, 128], bf16)
        nc.vector.memset(ones1[:], 1.0)
        iota = const.tile([128, 1], f32)
        nc.scalar.activation(iota[:], iota_bv, AF.Copy)

        # w as bf16 pairs: x2b holds w[128k+m]; store to DRAM once
        # (20 KB), broadcast-load [128, NV] bf16 (2.56 MB on SWDGE).
        # The gather then uses d=2 pairs (bf16 needs d*size%4==0).
        dpool = ctx.enter_context(
            tc.tile_pool(name="dram", bufs=1, space="DRAM"))
        wtab = dpool.tile([1, 80 * 128], bf16)
        wt = wtab[:]
        nc.gpsimd.dma_start(
            out=bass.AP(tensor=wt.tensor, offset=wt.offset,
                        ap=[[128, 80], [1, 128]]),
            in_=x2b[:])
        tabb = const.tile([128, NV], bf16)
        q = NV // 4
        for c in range(4):
            nc.gpsimd.dma_start(
                out=tabb[:, c * q:(c + 1) * q],
                in_=bass.AP(tensor=wt.tensor, offset=wt.offset + c * q,
                            ap=[[0, 128], [1, q]]))

        # ---- GP gathers issued early in the gpsimd stream ----------
        gp_z = []
        for gi, (ci, c) in enumerate(GP_CHUNKS):
            cpg, lpc, lpc_pad = GP_GEOM[gi]
            z = const.tile([128, 2 * lpc_pad], bf16, tag=f"z{gi}",
                           name=f"z{gi}")
            nc.gpsimd.ap_gather(
                z[:], tabb[:],
                pidx[:, GP_COL_OFFS[gi]:GP_COL_OFFS[gi] + GP_IDX_COLS[gi]],
                channels=128, num_elems=NV // 2, d=2, num_idxs=lpc_pad)
            gp_z.append(z)

        rings = [nc.sync, nc.scalar]

        def write_out(bcast, ci, cols):
            bt = bcast[:]
            prow = bt.ap[0][0]
            bap = bass.AP(tensor=bt.tensor, offset=bt.offset,
                          ap=[[prow, 128], [1, cols]])
            last = ci == len(PLAN) - 1
            for blk in range(8):
                dst = bass.AP(tensor=out_d,
                              offset=blk * 128 * C_PAD + COL_OFFS[ci],
                              ap=[[C_PAD, 128], [1, cols]])
                eng = (nc.gpsimd if last and blk in (3, 7)
                       else rings[blk % 2])
                eng.dma_start(out=dst, in_=bap)

        # pending tensor-tail from the previous chunk (bcast matmuls +
        # ACT copy + DMAs), emitted after the next chunk's first
        # tensor ops so the PE never idles at chunk boundaries
        pending = []

        def flush_pending():
            while pending:
                pending.pop(0)()

        gtile = 0
        pi = 0
        gi = 0
        for ci, (kind, c) in enumerate(PLAN):
            if kind == 'pe':
                nt = _ntiles(c)
                ohs = []
                yms = []

                def stage1(t, ohs=ohs, yms=yms):
                    nc.tensor.matmul(Y[:], x2b[:], ohs[t][0:80, 0:512],
                                     start=True, stop=True)
                    ym = ymp.tile([128, 512], bf16, tag="ym")
                    nc.vector.tensor_tensor(ym[:], Y[:],
                                            ohs[t][:, 512:1024], OP.mult)
                    yms.append(ym)

                for t in range(nt):
                    hirow = perows[0:1, 1024 * gtile:1024 * gtile + 512]
                    lorow = perows[0:1,
                                   1024 * gtile + 512:1024 * (gtile + 1)]
                    gtile += 1
                    PP = PPs[t % 2]
                    nc.tensor.matmul(PP[:, 0:512], ones1[:], hirow,
                                     start=True, stop=True)
                    nc.tensor.matmul(PP[:, 512:1024], ones1[:], lorow,
                                     start=True, stop=True)
                    if t == 0:
                        flush_pending()
                    oh = work.tile([128, 1024], bf16, tag="oh")
                    nc.vector.tensor_scalar(oh[:], PP[:], iota[:, 0:1],
                                            None, OP.is_equal)
                    ohs.append(oh)
                    if t >= 1:
                        stage1(t - 1)
                stage1(nt - 1)
                for t in range(nt):
                    nc.tensor.matmul(zP[:], paux_mat(PAUX_CSEL + 128 * t),
                                     yms[t][:],
                                     start=(t == 0), stop=(t == nt - 1))
                zsg = work.tile([4, 512], f32, tag="zsg")
                nc.vector.tensor_tensor(
                    zsg[0:nt, :], zP[0:nt, :],
                    smalls[0:nt, 512 * pi:512 * (pi + 1)], OP.mult)
                zs = work.tile([4, 512], f32, tag="zs")
                nc.scalar.activation(zs[0:nt, :], zsg[0:nt, :],
                                     AF.Sigmoid)
                p01 = work.tile([4, TILE_C], f32, tag="pp01")
                nc.vector.tensor_tensor(p01[0:nt, :], zs[0:nt, 0:504:3],
                                        zs[0:nt, 1:504:3], OP.mult)
                r2 = work.tile([4, TILE_C], bf16, tag="pr2")
                nc.vector.scalar_tensor_tensor(r2[0:nt, :], p01[0:nt, :],
                                               1.0, zs[0:nt, 2:504:3],
                                               OP.mult, OP.mult)
                rsel_o = len(PE_CHUNKS) * 512

                def tail(ci=ci, c=c, nt=nt, r2=r2):
                    for t in range(nt):
                        nc.tensor.matmul(
                            PA[:, t, 0:TILE_C],
                            smalls[0:nt, rsel_o + 128 * t:
                                   rsel_o + 128 * (t + 1)],
                            r2[0:nt, :], start=True, stop=True)
                    bcast = const.tile([128, c], f16, tag=f"bc{ci}")
                    bt = bcast[:]
                    pav = PA[:]
                    nc.scalar.activation(
                        bass.AP(tensor=bt.tensor, offset=bt.offset,
                                ap=[[bt.ap[0][0], 128], [TILE_C, nt],
                                    [1, TILE_C]]),
                        bass.AP(tensor=pav.tensor, offset=pav.offset,
                                ap=[[pav.ap[0][0], 128], [256, nt],
                                    [1, TILE_C]]),
                        AF.Copy, scale=-1.0, bias=1.0)
                    write_out(bcast, ci, c)
                pending.append(tail)
                pi += 1
            else:
                cpg, lpc, lpc_pad = GP_GEOM[gi]
                z = gp_z[gi]
                o = sum(lp for _, _, lp in GP_GEOM[:gi])
                flush_pending()
                par_v = pidx[:, PIDX_PAR + o:PIDX_PAR + o + lpc_pad]
                zsel = work.tile([128, lpc_pad], bf16, tag="gzsel")
                nc.vector.select(zsel[:], par_v, z[:, 1:2 * lpc_pad:2],
                                 z[:, 0:2 * lpc_pad:2])
                zsg = work.tile([128, lpc_pad], f32, tag="gzsg")
                nc.vector.tensor_tensor(
                    zsg[:], zsel[:],
                    bass.AP(tensor=pt.tensor,
                            offset=pt.offset + PAUX_SGN + o,
                            ap=[[prow_x, 128], [1, lpc_pad]]), OP.mult)
                zs = work.tile([128, lpc_pad], f32, tag="gzs")
                nc.scalar.activation(zs[:], zsg[:], AF.Sigmoid)
                p01 = work.tile([128, cpg], f32, tag="gp01")
                nc.vector.tensor_tensor(p01[:], zs[:, 0:lpc:3],
                                        zs[:, 1:lpc:3], OP.mult)
                r = work.tile([128, cpg], bf16, tag="gr")
                nc.vector.scalar_tensor_tensor(r[:], p01[:], 1.0,
                                               zs[:, 2:lpc:3],
                                               OP.mult, OP.mult)
                bcast = const.tile([128, c], f16, tag=f"bc{ci}")
                bt = bcast[:]
                prow = bt.ap[0][0]
                pav = PA[:]
                for half in range(2):
                    for g4 in range(4):
                        g = 4 * half + g4
                        nc.tensor.matmul(PA[:, g4, 0:cpg],
                                         paux_mat(PAUX_SEL + 128 * g),
                                         r[:], start=True, stop=True)
                    nc.scalar.activation(
                        bass.AP(tensor=bt.tensor,
                                offset=bt.offset + half * 4 * cpg,
                                ap=[[prow, 128], [cpg, 4], [1, cpg]]),
                        bass.AP(tensor=pav.tensor, offset=pav.offset,
                                ap=[[pav.ap[0][0], 128], [256, 4],
                                    [1, cpg]]),
                        AF.Copy, scale=-1.0, bias=1.0)
                write_out(bcast, ci, c)
                gi += 1
        flush_pending()
    nc.compile()
    return nc


def _prep_inputs(clause_idx, clause_sign):
    import ml_dtypes
    bf = ml_dtypes.bfloat16
    idx_all = clause_idx.astype(np.int32)
    # product factor per literal is (1 - y) = sigmoid(-sign * w): the
    # sign fed to the device sigmoid is the NEGATED clause sign
    sgn_all = np.where(clause_sign > 0.0, np.float32(-1.0),
                       np.float32(1.0))

    # shared constant blocks (same every core)
    k = np.arange(128)
    sel = (((k[:, None] // 16) == np.arange(GROUPS)[None, :])
           .astype(np.float32)[:, :, None]
           * np.full((1, 1, 128), 1.0 / 16.0, np.float32))
    sel = np.ascontiguousarray(np.broadcast_to(sel, (128, GROUPS, 128)))
    colsel = np.zeros((128, 4, 128), dtype=np.float32)
    for t in range(4):
        colsel[:, t, t] = 1.0
    rowsel = np.zeros((4, 4, 128), dtype=np.float32)
    for t in range(4):
        rowsel[t, t, :] = 1.0
    iota = np.arange(128, dtype=np.float32)

    per_core = []
    for cc in range(NCORES):
        cl_i = np.zeros((C_PAD, KLIT), dtype=np.int32)
        cl_s = np.ones((C_PAD, KLIT), dtype=np.float32)
        cl_i[:C_CORE] = idx_all[cc * C_CORE:(cc + 1) * C_CORE]
        cl_s[:C_CORE] = sgn_all[cc * C_CORE:(cc + 1) * C_CORE]

        perows = np.zeros((1, PE_TILES_TOT * 1024), dtype=np.float32)
        smalls = np.ones((4, len(PE_CHUNKS) * 512 + 512),
                         dtype=np.float32)
        smalls[:, len(PE_CHUNKS) * 512:] = rowsel.reshape(4, 512)
        idxw = np.zeros((128, IDX_COLS + SGN_TOT), dtype=np.int16)
        sgnz = np.ones((128, SGN_TOT), dtype=np.float32)

        gtile = 0
        pi = 0
        gi = 0
        for ci, (kind, c) in enumerate(PLAN):
            c0 = COL_OFFS[ci]
            if kind == 'pe':
                nt = _ntiles(c)
                for t in range(nt):
                    ncl = min(TILE_C, c - TILE_C * t)
                    ii = np.zeros((TILE_C, KLIT), dtype=np.int32)
                    ss = np.ones((TILE_C, KLIT), dtype=np.float32)
                    ii[:ncl] = cl_i[c0 + TILE_C * t:c0 + TILE_C * t + ncl]
                    ss[:ncl] = cl_s[c0 + TILE_C * t:c0 + TILE_C * t + ncl]
                    lits = ii.reshape(-1)
                    srow = ss.reshape(-1)
                    hi = (lits // RADIX).astype(np.float32)
                    lo = (lits % RADIX).astype(np.float32)
                    perows[0, 1024 * gtile:1024 * gtile + 504] = hi
                    perows[0, 1024 * gtile + 512:1024 * gtile + 1016] = lo
                    smalls[t, 512 * pi:512 * pi + 504] = srow
                    gtile += 1
                pi += 1
            else:
                cpg, lpc, lpc_pad = GP_GEOM[gi]
                blk_i = cl_i[c0:c0 + c].reshape(GROUPS, lpc)
                blk_s = cl_s[c0:c0 + c].reshape(GROUPS, lpc)
                gs_i = np.zeros((GROUPS, lpc_pad), dtype=np.int32)
                gs_s = np.ones((GROUPS, lpc_pad), dtype=np.float32)
                gs_i[:, :lpc] = blk_i
                gs_s[:, :lpc] = blk_s
                wi = ((gs_i >> 1).reshape(GROUPS, lpc_pad // 16, 16)
                      .transpose(0, 2, 1).reshape(128, lpc_pad // 16))
                idxw[:, GP_COL_OFFS[gi]:GP_COL_OFFS[gi] +
                     GP_IDX_COLS[gi]] = wi
                o = sum(lp for _, _, lp in GP_GEOM[:gi])
                sgnz[:, o:o + lpc_pad] = np.repeat(
                    gs_s[:, None, :], 16, axis=1).reshape(128, lpc_pad)
                idxw[:, PIDX_PAR + o:PIDX_PAR + o + lpc_pad] = \
                    np.repeat((gs_i & 1).astype(np.int16)[:, None, :],
                              16, axis=1).reshape(128, lpc_pad)
                gi += 1

        paux = np.zeros((128, PAUX_COLS), dtype=np.float32)
        paux[:, PAUX_SEL:PAUX_CSEL] = sel.reshape(128, -1)
        paux[:, PAUX_CSEL:PAUX_SGN] = colsel.reshape(128, -1)
        paux[:, PAUX_SGN:PAUX_IOTA] = sgnz
        paux[:, PAUX_IOTA] = iota

        per_core.append({
            "paux": paux.astype(bf),
            "pidx": idxw,
            "perows": perows.astype(bf),
            "smalls": smalls.astype(bf),
        })
    return per_core


def _ensure_ntff_hook():
    """The agent image lacks antenv.axon_hooks; synthesize it so
    run_bass_kernel_spmd(trace=True) can capture NTFF profiles."""
    import sys, types
    try:
        from antenv import axon_hooks  # noqa: F401
        return
    except ImportError:
        pass
    m = types.ModuleType("antenv.axon_hooks")
    _hook = [None]
    m.set_axon_ntff_profile_hook = lambda h: _hook.__setitem__(0, h)
    m.get_axon_ntff_profile_hook = lambda: _hook[0]
    sys.modules["antenv.axon_hooks"] = m
    import antenv
    antenv.axon_hooks = m
    from trn_agent_boot.trn_boot import _ntff_profile_via_ctypes
    m.set_axon_ntff_profile_hook(
        _ntff_profile_via_ctypes("/opt/axon/libaxon_pjrt.so"))


def _run(emb, per_core, trace=False):
    from concourse.bass_utils import run_bass_kernel_spmd
    if trace:
        _ensure_ntff_hook()
    if "prog" not in _CACHE:
        _CACHE["prog"] = _build()
    nc = _CACHE["prog"]
    in_maps = [{"emb": emb, **per_core[c]} for c in range(NCORES)]
    return run_bass_kernel_spmd(nc, in_maps, list(range(NCORES)),
                                trace=trace)


def kernel(input_idx=None, emb_weight=None, clause_idx=None,
           clause_sign=None, _trace=False, _want_results=False):
    emb = np.ascontiguousarray(np.asarray(emb_weight, dtype=np.float32))
    cidx = np.asarray(clause_idx, dtype=np.int32)
    csgn = np.asarray(clause_sign, dtype=np.float32)
    per_core = _prep_inputs(cidx, csgn)
    res = _run(emb, per_core, trace=_trace)
    full = np.empty((B, C_TOTAL), dtype=np.float32)
    for c in range(NCORES):
        full[:, c * C_CORE:(c + 1) * C_CORE] = \
            res.results[c]["out"][:, :C_CORE].astype(np.float32)
    if _want_results:
        return full, res
    return full
